# revision 32
# baseline (speedup 1.0000x reference)
"""FDN reverb kernel for 8x TRN2 NeuronCores.

Computes out = y / max|y| with y[t] = x[t] + sum_n a_n * x[t - d_n],
where a_n = (sum_j Q[j, n]) * g[n]  (the MIX=0.5 factor cancels in the
normalization).

Sharding: time axis split into 8 contiguous shards of 1M samples; each
core's input carries a max-delay halo from the previous shard (zeros for
core 0).  On-core layout is partition-major: partition p holds samples
[p*F, p*F + F) of the shard plus a D-sample halo in front, so every
delayed read is a free-axis offset.

All data is fp16 (measured end-to-end rel err ~1.5e-3 vs the fp32
reference).  Per 1024-col chunk the 8 delay taps are split across
engines: 6 taps run on the tensor engine as diagonal-stationary matmuls
accumulating in PSUM, which the scalar engine evacuates to fp16 y; 2
taps + the identity run on the DVE as tensor_scalar (4x mode) +
tensor_tensor (2x mode) pairs into y_v, which a gpsimd-initiated
SBUF->SBUF DMA accumulates into y (accum_op=add, CCE inline ALU); the
DVE folds a running abs-max.  A tiny AllGather(max) yields the global
normalizer; DVE/ACT scale; DMA out.
"""

import numpy as np

import concourse.bacc as bacc
import concourse.bass as bass
import concourse.mybir as mybir
import concourse.tile as tile
from concourse.bass_utils import run_bass_kernel_spmd

# ---- problem constants (hardcoded; must match the reference) ----
SAMPLE_RATE = 48000
DELAYS_SEC = [0.0297, 0.0371, 0.0411, 0.0437, 0.0533, 0.0617, 0.0731, 0.0797]
DELAYS = [int(d * SAMPLE_RATE) for d in DELAYS_SEC]  # [1425,...,3825]
NTAPS = len(DELAYS)  # 8
T = 8388608
N_CORES = 8
T_CORE = T // N_CORES  # 1048576
P = 128
F = T_CORE // P  # 8192 samples per partition row
D = 3840  # halo (>= max delay 3825), 128-aligned
CH = 1024  # processing chunk (free dim)
NCH = F // CH  # 8
HT = 512  # PSUM bank tile / matmul moving width

# tap split: big delays on PE (early columns -> PE starts first),
# the two smallest + identity on the DVE; the final chunk runs all-PE
PE_DELAYS = [3825, 3508, 2961, 2558, 2097, 1972]
DVE_DELAYS = [1780, 1425]
MERGE_DMA = True  # merge y_v into y via gpsimd accum-DMA (else PE matmul)
NWARM = 8  # PE p-state warmup matmuls

# in-DMA column chunks of the [128, D+F] overlapped row view
XBOUNDS = [0, 640] + [640 + 1424 * (k + 1) for k in range(8)]  # ... 12032

# out-DMA / scale chunks (first/last small so the tail pipeline starts fast)
SBOUNDS = [0, 512] + [512 + 1024 * (k + 1) for k in range(7)] + [8192]

_cache = {}


def _build_nc():
    fp32 = mybir.dt.float32
    fp16 = mybir.dt.float16
    u16 = mybir.dt.uint16
    nblk = len(PE_DELAYS) + 3  # 6 diagonals + identity + 2 small-tap diagonals

    nc = bacc.Bacc(
        "TRN2",
        target_bir_lowering=False,
        debug=False,
        enable_asserts=False,
        num_devices=N_CORES,
    )

    xh_d = nc.dram_tensor("xh", [1, D + T_CORE], fp16, kind="ExternalInput")
    # stationaries: tap diagonals (+ identity for the merge if on PE), fp16
    wmat = nc.dram_tensor("wmat", [P, nblk * P], fp16, kind="ExternalInput")
    # full-precision per-partition coeff scalars for the DVE taps
    avec = nc.dram_tensor("avec", [P, 8], fp32, kind="ExternalInput")
    out = nc.dram_tensor("out", [1, T_CORE], fp16, kind="ExternalOutput")

    def shard_ap(t, c0, c1):
        # columns [c0, c1) of the overlapped [128, D+F] row view
        return bass.AP(tensor=t, offset=c0, ap=[[F, P], [1, c1 - c0]])

    with tile.TileContext(nc) as tc:
        with (
            tc.tile_pool(name="xpool", bufs=1) as xpool,
            tc.tile_pool(name="ypool", bufs=1) as ypool,
            tc.tile_pool(name="dpool", bufs=1) as dpool,
            tc.tile_pool(name="vpool", bufs=4) as vpool,
            tc.tile_pool(name="tpool", bufs=4) as tpool,
            tc.tile_pool(name="spool", bufs=1) as spool,
            tc.tile_pool(name="psum", bufs=7, space="PSUM") as psum_pool,
            tc.tile_pool(name="psumt", bufs=1, space="PSUM") as psumt_pool,
            tc.tile_pool(name="dram", bufs=1, space="DRAM") as dram_pool,
        ):
            x_t = xpool.tile([P, D + F], fp16)
            y_t = ypool.tile([P, F], fp16)
            w_t = dpool.tile([P, nblk * P], fp16)
            av_t = dpool.tile([P, 8], fp32, name="av_t")
            st = spool.tile([P, 16], fp32, name="st")
            stu = spool.tile([1, 8], u16, name="stu")
            su1 = spool.tile([1, 1], u16, name="su1")
            m_loc = spool.tile([P, 1], fp32)
            cc_sb = spool.tile([1, 8], fp32)
            g_all = spool.tile([P, 8 * N_CORES], fp32)
            inv_b = spool.tile([P, 1], fp32)

            cc_in = dram_pool.tile([1, 8], fp32)
            cc_out = dram_pool.tile([N_CORES, 8], fp32, addr_space="Shared")

            jtile = dpool.tile([P, P], fp16, name="jtile")
            nc.vector.memset(jtile[:], 1.0)
            # PE p-state warmup: junk matmuls on the memset tile (no DMA dep)
            pwarm = psumt_pool.tile([P, P], fp32, tag="pt", name="pwarm")
            for _ in range(NWARM):
                nc.tensor.matmul(
                    pwarm[:], jtile[:], jtile[:], start=True, stop=True,
                )

            nc.sync.dma_start(w_t[:], wmat.ap())
            for i, (c0, c1) in enumerate(zip(XBOUNDS[:-1], XBOUNDS[1:])):
                nc.sync.dma_start(x_t[:, c0:c1], shard_ap(xh_d, c0, c1))
                if i == 1:
                    nc.sync.dma_start(av_t[:], avec.ap())

            nc.vector.memset(cc_sb[:], 0.0)
            nc.vector.memset(st[:], 0.0)

            d0, d1 = DVE_DELAYS

            def emit_yv(j):
                # DVE: 2 taps via tensor_scalar (4x) + tensor_tensor (2x),
                # identity folded into the first add's second operand
                b = D + j * CH
                yv = vpool.tile([P, CH], fp16, tag="yv", name=f"yv_{j}")
                t0 = tpool.tile([P, CH], fp16, tag="ts", name=f"t0_{j}")
                nc.vector.tensor_scalar_mul(
                    t0[:], x_t[:, b - d0 : b - d0 + CH], av_t[:, 0:1]
                )
                nc.vector.tensor_tensor(
                    yv[:], t0[:], x_t[:, b : b + CH], op=mybir.AluOpType.add
                )
                t1 = tpool.tile([P, CH], fp16, tag="ts", name=f"t1_{j}")
                nc.vector.tensor_scalar_mul(
                    t1[:], x_t[:, b - d1 : b - d1 + CH], av_t[:, 1:2]
                )
                nc.vector.tensor_tensor(
                    yv[:], yv[:], t1[:], op=mybir.AluOpType.add
                )
                return yv

            yv_q = [emit_yv(0), emit_yv(1)]
            for j in range(NCH):
                base = D + j * CH
                c0 = j * CH
                pe_merge = j >= NCH - 2  # last two chunks merge on the PE
                last = j == NCH - 1
                yv = yv_q.pop(0)

                # the final chunk tapers 512/256/256 so the drain pipeline
                # (evac -> abs-reduce) gets short tail pieces
                widths = [HT // 2] * 4 if last else [HT, HT]
                off = 0
                for h, wd in enumerate(widths):
                    hb = base + off
                    ps = psum_pool.tile([P, HT], fp32, tag="ps", name=f"ps_{j}_{h}")
                    for t_i, dd in enumerate(PE_DELAYS):
                        nc.tensor.matmul(
                            ps[:, :wd],
                            w_t[:, t_i * P : (t_i + 1) * P],
                            x_t[:, hb - dd : hb - dd + wd],
                            start=(t_i == 0),
                            stop=(not pe_merge)
                            and (t_i == len(PE_DELAYS) - 1),
                        )
                    if pe_merge:
                        nc.tensor.matmul(
                            ps[:, :wd],
                            w_t[:, 6 * P : 7 * P],
                            yv[:, off : off + wd],
                            start=False, stop=True,
                        )
                    nc.scalar.copy(y_t[:, c0 + off : c0 + off + wd], ps[:, :wd])
                    if last:
                        nc.vector.tensor_reduce(
                            st[:, NCH - 1 + h : NCH + h],
                            y_t[:, c0 + off : c0 + off + wd],
                            axis=mybir.AxisListType.X, op=mybir.AluOpType.max,
                            apply_absolute_value=True,
                        )
                    off += wd

                if j + 2 < NCH:
                    yv_q.append(emit_yv(j + 2))
                if not pe_merge:
                    nc.gpsimd.dma_start(
                        y_t[:, c0 : c0 + CH], yv[:],
                        accum_op=mybir.AluOpType.add,
                    )
                # per-chunk |max| stats: odd dma-merged chunks compute |y|
                # cheaply on the DVE (bitwise-and on the u16 view, 4x mode)
                # and fold on gpsimd as a u16 cross-partition max; the rest
                # reduce with absolute on the DVE; final chunk in the drain
                if j in (1, 3, 5):
                    ay = tpool.tile([P, CH], fp16, tag="ay", name=f"ay_{j}")
                    nc.vector.tensor_scalar(
                        ay[:].bitcast(u16), y_t[:, c0 : c0 + CH].bitcast(u16),
                        0x7FFF, None, op0=mybir.AluOpType.bitwise_and,
                    )
                    nc.gpsimd.tensor_reduce(
                        stu[0:1, (j - 1) // 2 : (j + 1) // 2], ay[:].bitcast(u16),
                        axis=mybir.AxisListType.XYZWC, op=mybir.AluOpType.max,
                    )
                    if j == 5:
                        nc.gpsimd.tensor_reduce(
                            su1[0:1, 0:1], stu[0:1, 0:3],
                            axis=mybir.AxisListType.XYZWC,
                            op=mybir.AluOpType.max,
                        )
                elif j < NCH - 1:
                    nc.vector.tensor_reduce(
                        st[:, j : j + 1], y_t[:, c0 : c0 + CH],
                        axis=mybir.AxisListType.X, op=mybir.AluOpType.max,
                        apply_absolute_value=True,
                    )

            # local max: fold stats columns, gpsimd partition fold; the u16
            # row was folded early (hidden under the stream)
            nc.vector.tensor_reduce(
                m_loc[:, 0:1], st[:, 0 : NCH + 3],
                axis=mybir.AxisListType.X, op=mybir.AluOpType.max,
            )
            nc.gpsimd.tensor_reduce(
                cc_sb[0:1, 0:1], m_loc[:, 0:1], axis=mybir.AxisListType.XYZWC,
                op=mybir.AluOpType.max,
            )
            nc.vector.tensor_copy(
                cc_sb[0:1, 1:2], su1[0:1, 0:1].bitcast(fp16)
            )
            # global max across cores
            nc.sync.dma_start(cc_in[:], cc_sb[:])
            nc.gpsimd.collective_compute(
                "AllGather",
                mybir.AluOpType.bypass,
                replica_groups=[list(range(N_CORES))],
                ins=[cc_in[:].opt()],
                outs=[cc_out[:].opt()],
            )
            nc.sync.dma_start(
                g_all[:],
                bass.AP(tensor=cc_out.tensor, offset=0, ap=[[0, P], [1, 8 * N_CORES]]),
            )
            nc.vector.tensor_reduce(
                inv_b[:], g_all[:], axis=mybir.AxisListType.X, op=mybir.AluOpType.max
            )
            nc.vector.reciprocal(inv_b[:], inv_b[:])

            # scale + store, DVE/ACT alternating, DMA out per chunk
            for i, (c0, c1) in enumerate(zip(SBOUNDS[:-1], SBOUNDS[1:])):
                ysl = y_t[:, c0:c1]
                if i % 2 == 0:
                    nc.vector.tensor_scalar_mul(ysl, ysl, inv_b[:, 0:1])
                else:
                    nc.scalar.mul(ysl, ysl, inv_b[:, 0:1])
                nc.sync.dma_start(shard_ap(out, c0, c1), ysl)

    nc.compile()
    return nc


def _prep_inputs(input_sig, feedback_gain, orthogonal_matrix):
    x = np.ascontiguousarray(np.asarray(input_sig, dtype=np.float32)).reshape(T)
    g = np.asarray(feedback_gain, dtype=np.float32)
    q = np.asarray(orthogonal_matrix, dtype=np.float32)
    coeff = (q.sum(axis=0) * g).astype(np.float32)  # [8]
    di = {dd: i for i, dd in enumerate(DELAYS)}
    nblk = len(PE_DELAYS) + 3

    xpad = np.concatenate([np.zeros(D, np.float32), x]).astype(np.float16)
    idx = np.arange(P)

    wmat = np.zeros((P, nblk * P), dtype=np.float16)
    for t_i, dd in enumerate(PE_DELAYS):
        wmat[idx, t_i * P + idx] = coeff[di[dd]].astype(np.float16)
    wmat[idx, 6 * P + idx] = np.float16(1.0)
    for bi, dd in ((7, DVE_DELAYS[0]), (8, DVE_DELAYS[1])):
        wmat[idx, bi * P + idx] = coeff[di[dd]].astype(np.float16)

    avec = np.zeros((P, 8), dtype=np.float32)
    for i, dd in enumerate(DVE_DELAYS):
        avec[:, i] = coeff[di[dd]]

    in_maps = []
    for c in range(N_CORES):
        sl = slice(c * T_CORE, c * T_CORE + D + T_CORE)
        in_maps.append({
            "xh": np.ascontiguousarray(xpad[sl]).reshape(1, D + T_CORE),
            "wmat": wmat,
            "avec": avec,
        })
    return in_maps


def _run(in_maps, trace=False):
    if "nc" not in _cache:
        _cache["nc"] = _build_nc()
    nc = _cache["nc"]
    res = run_bass_kernel_spmd(
        nc, in_maps, core_ids=list(range(N_CORES)), trace=trace
    )
    outs = [r["out"].reshape(T_CORE).astype(np.float32) for r in res.results]
    full = np.concatenate(outs).reshape(1, T)
    return full, res


def kernel(input_sig, feedback_gain, orthogonal_matrix):
    in_maps = _prep_inputs(input_sig, feedback_gain, orthogonal_matrix)
    try:
        full, _ = _run(in_maps, trace=False)
    except Exception:
        # one retry: a freshly-attached terminal occasionally reports a
        # transient device-unrecoverable error on the first execution
        full, _ = _run(in_maps, trace=False)
    return full
